# revision 2
# baseline (speedup 1.0000x reference)
"""Trainium2 Bass kernel v2 for the snntorch-style 2-layer spiking net.

Same numerics as the baseline (fp16+fp8 split x, 3-pass layer-1, sign trick,
bf16 hi/lo layer-2), restructured schedule:

  - Layer-1 chunk matmuls batched over 2 timesteps (N=512 moving operand,
    chunk-major input packing) -> half the LDWEIGHTS + instruction overhead.
  - Layer-1 reset moved off the PE onto the DVE (q = -0.5*sign_prev + p1),
    which also removes the PE<-ACT dependency cycle: the PE stream depends
    only on DMA'd inputs and 1+-step-old sign1 history (layer-2 only).
  - Layer-2 batched per 2-step group: 2 matmuls of N=512 vs 4 of N=256.
  - mem2 scan (u2/m2/s2) deferred 4 steps behind the mem1 chain so the DVE
    FIFO never head-of-line blocks on layer-2 PSUM.
  - 3 large DMAs per 8-step block (3.1/1.6/0.2 MB contiguous).
  - PSUM: 2-step p1 tiles [128,512] (1 bank) x 5 + p2 [10,512] x 2.

Outputs identical to baseline: m2out [10,T,256] f32, s2out [10,T,256] bf16.
"""
import contextlib

import numpy as np
import ml_dtypes

import concourse.bass as bass
import concourse.tile as tile
from concourse import bacc, mybir
from concourse import bass_utils

N_CORES = 8
B, T, NIN, NH, NOUT = 2048, 201, 784, 128, 10
BS = B // N_CORES          # 256
TB = 8                     # block = 8 steps
NBLK = T // TB             # 25 full blocks; step 200 is the tail
NFULL = 6
REM = NIN - NFULL * 128    # 16
KREM = REM + 3 + REM + REM # 51
BETA = 0.9
THR = 1.0
NG = (T + 1) // 2          # 101 2-step groups (last has 1 step)
LA = 4                     # p1 group lookahead
SHIFT = 4                  # mem2-scan lag in steps

BF16 = ml_dtypes.bfloat16
FP8 = ml_dtypes.float8_e4m3fn
XLS = 4096.0


def gsteps(g):
    t0 = 2 * g
    return [t0] if t0 == T - 1 else [t0, t0 + 1]


def build_kernel(reps_loop=False, l2_onepass=False):
    hidt = mybir.dt.float16
    lodt = mybir.dt.float8e4

    nc = bacc.Bacc("TRN2", target_bir_lowering=False, debug=False,
                   num_devices=N_CORES)

    xh = nc.dram_tensor("xh", [128, NBLK, NFULL, TB, BS], hidt,
                        kind="ExternalInput").ap()
    xl = nc.dram_tensor("xl", [128, NBLK, NFULL, TB, BS], lodt,
                        kind="ExternalInput").ap()
    xht = nc.dram_tensor("xht", [128, NFULL, BS], hidt,
                         kind="ExternalInput").ap()
    xlt = nc.dram_tensor("xlt", [128, NFULL, BS], lodt,
                         kind="ExternalInput").ap()
    xr = nc.dram_tensor("xr", [KREM, T, BS], hidt,
                        kind="ExternalInput").ap()
    wh = nc.dram_tensor("wh", [128, NFULL, NH], hidt,
                        kind="ExternalInput").ap()
    wl = nc.dram_tensor("wl", [128, NFULL, NH], mybir.dt.bfloat16,
                        kind="ExternalInput").ap()
    w8 = nc.dram_tensor("w8", [128, NFULL, NH], mybir.dt.bfloat16,
                        kind="ExternalInput").ap()
    wrem = nc.dram_tensor("wrem", [KREM, NH], hidt,
                          kind="ExternalInput").ap()
    w2 = nc.dram_tensor("w2", [NH, 2, NOUT], mybir.dt.bfloat16,
                        kind="ExternalInput").ap()
    b2e = nc.dram_tensor("b2e", [NOUT, 1], mybir.dt.float32,
                         kind="ExternalInput").ap()
    m2out = nc.dram_tensor("m2out", [NOUT, T, BS], mybir.dt.float32,
                           kind="ExternalOutput").ap()
    s2out = nc.dram_tensor("s2out", [NOUT, T, BS], mybir.dt.bfloat16,
                           kind="ExternalOutput").ap()
    if reps_loop:
        reps = nc.dram_tensor("reps", [1, 1], mybir.dt.int32,
                              kind="ExternalInput").ap()

    with tile.TileContext(nc) as tc:
        with tc.tile_pool(name="wpool", bufs=1) as wpool, \
             tc.tile_pool(name="xpool", bufs=3) as xpool, \
             tc.tile_pool(name="state", bufs=1) as state, \
             tc.tile_pool(name="hist", bufs=2) as hist, \
             tc.tile_pool(name="p1pool", bufs=LA + 1, space="PSUM") as p1pool, \
             tc.tile_pool(name="p2pool", bufs=2, space="PSUM") as p2pool:

            wh_t = wpool.tile([128, NFULL, NH], hidt)
            nc.sync.dma_start(wh_t[:], wh[:])
            wl_t = wpool.tile([128, NFULL, NH], mybir.dt.bfloat16)
            nc.sync.dma_start(wl_t[:], wl[:])
            w8_t = wpool.tile([128, NFULL, NH], mybir.dt.bfloat16)
            nc.sync.dma_start(w8_t[:], w8[:])
            wrem_t = wpool.tile([KREM, NH], hidt)
            nc.sync.dma_start(wrem_t[:], wrem[:])
            w2_t = wpool.tile([NH, 2, NOUT], mybir.dt.bfloat16)
            nc.sync.dma_start(w2_t[:], w2[:])
            b2e_t = wpool.tile([NOUT, 1], mybir.dt.float32)
            nc.sync.dma_start(b2e_t[:], b2e[:])
            biasm1 = wpool.tile([NH, 1], mybir.dt.float32)
            nc.gpsimd.memset(biasm1[:], -THR)
            xht_t = wpool.tile([128, NFULL, BS], hidt)
            nc.sync.dma_start(xht_t[:], xht[:])
            xlt_t = wpool.tile([128, NFULL, BS], lodt)
            nc.sync.dma_start(xlt_t[:], xlt[:])
            rem_tail = wpool.tile([KREM, 1, BS], hidt)
            nc.sync.dma_start(rem_tail[:], xr[:, T - 1:T, :])

            if reps_loop:
                rt = wpool.tile([1, 1], mybir.dt.int32)
                nc.sync.dma_start(rt[:], reps[:])
                regs = []
                for eng in (nc.tensor, nc.vector, nc.scalar, nc.gpsimd,
                            nc.sync):
                    r = eng.alloc_register(f"reps_{len(regs)}")
                    eng.reg_load(r, rt[0:1, 0:1])
                    regs.append(r)
                rv = nc.snap(bass.RegisterHandles(regs), min_val=0,
                             max_val=1 << 20)
                loop_cm = tc.For_i(0, rv, 1)
            else:
                loop_cm = contextlib.nullcontext()

            with loop_cm:
                sign1_init = state.tile([NH, BS], mybir.dt.bfloat16)
                nc.gpsimd.memset(sign1_init[:], -1.0)
                mem1_init = state.tile([NH, BS], mybir.dt.float32)
                nc.gpsimd.memset(mem1_init[:], 0.0)
                m2_init = state.tile([NOUT, BS], mybir.dt.float32)
                nc.gpsimd.memset(m2_init[:], 0.0)
                s2_init = state.tile([NOUT, BS], mybir.dt.bfloat16)
                nc.gpsimd.memset(s2_init[:], 0.0)

                xh_tiles = [None] * NBLK
                xl_tiles = [None] * NBLK
                rem_tiles = [None] * (NBLK + 1)
                rem_tiles[NBLK] = rem_tail

                def load_block(bi):
                    xh_b = xpool.tile([128, NFULL, TB, BS], hidt,
                                      name=f"xh_b{bi}", tag="xh_b")
                    xl_b = xpool.tile([128, NFULL, TB, BS], lodt,
                                      name=f"xl_b{bi}", tag="xl_b")
                    nc.sync.dma_start(xh_b[:], xh[:, bi, :, :, :])
                    nc.sync.dma_start(xl_b[:], xl[:, bi, :, :, :])
                    xh_tiles[bi] = xh_b
                    xl_tiles[bi] = xl_b
                    bt0 = bi * TB
                    rem_b = xpool.tile([KREM, TB, BS], hidt,
                                       name=f"rem_b{bi}", tag="rem_b")
                    nc.sync.dma_start(rem_b[:], xr[:, bt0:bt0 + TB, :])
                    rem_tiles[bi] = rem_b

                def group_mms(g, p1):
                    """chunks+rem for group g into p1 ([NH,512] or [NH,256])."""
                    ts = gsteps(g)
                    if len(ts) == 2:
                        bi = ts[0] // TB
                        ti = ts[0] % TB
                        xh_b, xl_b = xh_tiles[bi], xl_tiles[bi]
                        for c in range(NFULL):
                            nc.tensor.matmul(p1[:], wh_t[:, c, :],
                                             xh_b[:, c, ti:ti + 2, :],
                                             start=(c == 0), stop=False)
                        for c in range(NFULL):
                            nc.tensor.matmul(p1[:], wl_t[:, c, :],
                                             xh_b[:, c, ti:ti + 2, :],
                                             start=False, stop=False)
                        for c in range(NFULL):
                            nc.tensor.matmul(p1[:], w8_t[:, c, :],
                                             xl_b[:, c, ti:ti + 2, :],
                                             start=False, stop=False)
                        nc.tensor.matmul(p1[:], wrem_t[:],
                                         rem_tiles[bi][:, ti:ti + 2, :],
                                         start=False, stop=True)
                    else:
                        for c in range(NFULL):
                            nc.tensor.matmul(p1[:], wh_t[:, c, :],
                                             xht_t[:, c, :],
                                             start=(c == 0), stop=False)
                        for c in range(NFULL):
                            nc.tensor.matmul(p1[:], wl_t[:, c, :],
                                             xht_t[:, c, :],
                                             start=False, stop=False)
                        for c in range(NFULL):
                            nc.tensor.matmul(p1[:], w8_t[:, c, :],
                                             xlt_t[:, c, :],
                                             start=False, stop=False)
                        nc.tensor.matmul(p1[:], wrem_t[:],
                                         rem_tiles[NBLK][:, 0, :],
                                         start=False, stop=True)

                # ---------- prologue ----------
                load_block(0)
                load_block(1)

                p1_tiles = {}

                def alloc_p1(g):
                    n = len(gsteps(g))
                    p1 = p1pool.tile([NH, n * BS], mybir.dt.float32,
                                     name=f"p1_{g % (LA + 1)}", tag="p1")
                    p1_tiles[g] = p1
                    group_mms(g, p1)

                for g in range(LA):
                    alloc_p1(g)

                s1_tiles = {}
                s1_prev = (sign1_init, 0)    # (tile, col) of sign1(t-1)
                mem1_prev = mem1_init

                m2hist_prev, m2pcol = m2_init, 0
                s2hist_prev, s2pcol = s2_init, 0
                m2hist = s2hist = None
                p2_tiles = {}

                def do_l2(gp):
                    """Layer-2 matmuls for group gp (sign1 history ready)."""
                    n = len(gsteps(gp))
                    s1t = s1_tiles[gp]
                    p2 = p2pool.tile([NOUT, n * BS], mybir.dt.float32,
                                     name=f"p2_{gp % 2}", tag="p2")
                    p2_tiles[gp] = p2
                    nc.tensor.matmul(p2[:], w2_t[:, 0, :], s1t[:],
                                     start=True, stop=l2_onepass)
                    if not l2_onepass:
                        nc.tensor.matmul(p2[:], w2_t[:, 1, :], s1t[:],
                                         start=False, stop=True)

                def scan_step(t):
                    """mem2-scan for step t (runs SHIFT steps late)."""
                    nonlocal m2hist_prev, m2pcol, s2hist_prev, s2pcol
                    nonlocal m2hist, s2hist
                    bi, ti = t // TB, t % TB
                    bt0 = bi * TB
                    btb = min(TB, T - bt0)
                    if ti == 0:
                        m2hist = hist.tile([NOUT, btb * BS], mybir.dt.float32,
                                           name=f"m2h_{bi % 2}", tag="m2h")
                        s2hist = hist.tile([NOUT, btb * BS], mybir.dt.bfloat16,
                                           name=f"s2h_{bi % 2}", tag="s2h")
                    p2 = p2_tiles[t // 2]
                    hc = t % 2
                    u2 = state.tile([NOUT, BS], mybir.dt.float32,
                                    name=f"u2_{t % 2}", tag="u2", bufs=2)
                    nc.vector.scalar_tensor_tensor(
                        u2[:], s2hist_prev[:, s2pcol * BS:(s2pcol + 1) * BS],
                        b2e_t[:, 0:1], p2[:, hc * BS:(hc + 1) * BS],
                        mybir.AluOpType.subtract, mybir.AluOpType.subtract)
                    m2dst = m2hist[:, ti * BS:(ti + 1) * BS]
                    nc.vector.scalar_tensor_tensor(
                        m2dst, m2hist_prev[:, m2pcol * BS:(m2pcol + 1) * BS],
                        BETA, u2[:],
                        mybir.AluOpType.mult, mybir.AluOpType.subtract)
                    nc.vector.tensor_scalar(
                        s2hist[0:NOUT, ti * BS:(ti + 1) * BS], m2dst, THR,
                        None, mybir.AluOpType.is_gt)
                    m2hist_prev, m2pcol = m2hist, ti
                    s2hist_prev, s2pcol = s2hist, ti

                    th = max(btb // 2, 1)
                    if ti == th - 1 and btb > 1:
                        nc.scalar.dma_start(
                            m2out[:, bt0:bt0 + th, :],
                            m2hist[:, 0:th * BS]
                            .rearrange("o (t b) -> o t b", t=th))
                        nc.scalar.dma_start(
                            s2out[:, bt0:bt0 + th, :],
                            s2hist[0:NOUT, 0:th * BS]
                            .rearrange("o (t b) -> o t b", t=th))
                    elif ti == btb - 1:
                        lo = th * BS if btb > 1 else 0
                        tlo = bt0 + th if btb > 1 else bt0
                        ntt = bt0 + btb - tlo
                        nc.scalar.dma_start(
                            m2out[:, tlo:bt0 + btb, :],
                            m2hist[:, lo:btb * BS]
                            .rearrange("o (t b) -> o t b", t=ntt))
                        nc.scalar.dma_start(
                            s2out[:, tlo:bt0 + btb, :],
                            s2hist[0:NOUT, lo:btb * BS]
                            .rearrange("o (t b) -> o t b", t=ntt))

                # ---------- main loop over 2-step groups ----------
                for g in range(NG):
                    ts = gsteps(g)
                    if g % 4 == 0:
                        bi = g // 4 + 2
                        if bi < NBLK:
                            load_block(bi)
                    if g >= 1:
                        do_l2(g - 1)
                    if g + LA < NG:
                        alloc_p1(g + LA)

                    p1 = p1_tiles.pop(g)
                    s1t = state.tile([NH, len(ts) * BS], mybir.dt.bfloat16,
                                     name=f"s1_{g % 3}", tag="s1", bufs=3)
                    s1_tiles[g] = s1t
                    for k, t in enumerate(ts):
                        # q = -0.5*sign1(t-1) + p1[t]   (DVE, psum operand)
                        q = state.tile([NH, BS], mybir.dt.float32,
                                       name=f"q_{t % 2}", tag="q", bufs=2)
                        pt, pc = s1_prev
                        nc.vector.scalar_tensor_tensor(
                            q[:], pt[:, pc * BS:(pc + 1) * BS], -0.5,
                            p1[:, k * BS:(k + 1) * BS],
                            mybir.AluOpType.mult, mybir.AluOpType.add)
                        # mem1 = 0.9*mem1_prev + q
                        mem1 = state.tile([NH, BS], mybir.dt.float32,
                                          name=f"mem1_{t % 2}", tag="mem1",
                                          bufs=2)
                        nc.vector.scalar_tensor_tensor(
                            mem1[:], mem1_prev[:], BETA, q[:],
                            mybir.AluOpType.mult, mybir.AluOpType.add)
                        # sign1 = Sign(mem1 - 1) -> history slice (ACT)
                        nc.scalar.sign(s1t[:, k * BS:(k + 1) * BS],
                                       mem1[:], bias=biasm1[:])
                        mem1_prev = mem1
                        s1_prev = (s1t, k)
                        if t - SHIFT >= 0:
                            scan_step(t - SHIFT)

                # ---------- epilogue ----------
                # scan 197-199 BEFORE do_l2(100): its p2 buffer reuse would
                # otherwise overwrite p2(98) which scan(197) still reads
                for t in range(T - SHIFT, T - 1):
                    scan_step(t)
                do_l2(NG - 1)
                scan_step(T - 1)

    nc.compile()
    return nc


def _split3_f16(v):
    h = v.astype(np.float16)
    r = v - h.astype(np.float64)
    m = r.astype(np.float16)
    r2 = r - m.astype(np.float64)
    l = r2.astype(np.float16)
    return h, m, l


def prepare_inputs(x, W1, b1, W2, b2):
    """Host-side sharding + dtype splitting. Returns in_maps for 8 cores."""
    x = np.ascontiguousarray(x, dtype=np.float32)
    W1 = np.asarray(W1, dtype=np.float32)
    b1 = np.asarray(b1, dtype=np.float32)
    W2 = np.asarray(W2, dtype=np.float32)
    b2 = np.asarray(b2, dtype=np.float32)

    W1f = np.asarray(W1, np.float64)
    xh16 = x.astype(np.float16)
    xres = x - xh16.astype(np.float32)
    xl_pl = (xres * np.float32(XLS)).astype(FP8)
    xlr = xres[:, :, NFULL * 128:].astype(np.float16)
    W1a = W1f.astype(np.float16)
    W1b = (W1f - W1a.astype(np.float64)).astype(BF16)
    W18 = (W1f * (1.0 / XLS)).astype(BF16)
    b1h, b1m, b1l = _split3_f16(b1.astype(np.float64) - 0.5)
    wrem_mid = np.ascontiguousarray(W1b.T)[NFULL * 128:] \
        .astype(np.float64).astype(np.float16)

    W1aT = np.ascontiguousarray(W1a.T)
    wh = np.ascontiguousarray(
        W1aT[:NFULL * 128].reshape(NFULL, 128, NH).transpose(1, 0, 2))
    wl = np.ascontiguousarray(
        np.ascontiguousarray(W1b.T)[:NFULL * 128]
        .reshape(NFULL, 128, NH).transpose(1, 0, 2))
    w8 = np.ascontiguousarray(
        np.ascontiguousarray(W18.T)[:NFULL * 128]
        .reshape(NFULL, 128, NH).transpose(1, 0, 2))

    wrem = np.concatenate([
        W1aT[NFULL * 128:].astype(np.float64),
        b1h[None, :].astype(np.float64), b1m[None, :].astype(np.float64),
        b1l[None, :].astype(np.float64),
        wrem_mid.astype(np.float64),
        W1aT[NFULL * 128:].astype(np.float64),
    ], axis=0).astype(np.float16)
    assert wrem.shape == (KREM, NH)

    W2half = 0.5 * W2.astype(np.float64)
    W2hi = W2half.astype(BF16)
    W2lo = (W2half - W2hi.astype(np.float64)).astype(BF16)
    w2 = np.stack([np.ascontiguousarray(W2hi.T),
                   np.ascontiguousarray(W2lo.T)], axis=1)
    b2eff = (b2.astype(np.float64) + W2half.sum(axis=1)).astype(np.float32)
    b2e = np.ascontiguousarray(b2eff[:, None])

    in_maps = []
    for c in range(N_CORES):
        sl = slice(c * BS, (c + 1) * BS)
        xh_full = xh16[sl].transpose(2, 1, 0)          # [784, T, 256]
        xl_full = xl_pl[sl].transpose(2, 1, 0)
        xh_c = np.ascontiguousarray(
            xh_full[:NFULL * 128, :NBLK * TB]
            .reshape(NFULL, 128, NBLK, TB, BS).transpose(1, 2, 0, 3, 4))
        xl_c = np.ascontiguousarray(
            xl_full[:NFULL * 128, :NBLK * TB]
            .reshape(NFULL, 128, NBLK, TB, BS).transpose(1, 2, 0, 3, 4))
        xht_c = np.ascontiguousarray(
            xh_full[:NFULL * 128, T - 1]
            .reshape(NFULL, 128, BS).transpose(1, 0, 2))
        xlt_c = np.ascontiguousarray(
            xl_full[:NFULL * 128, T - 1]
            .reshape(NFULL, 128, BS).transpose(1, 0, 2))
        xr_c = np.empty((KREM, T, BS), np.float16)
        xr_c[0:REM] = xh_full[NFULL * 128:]
        xr_c[REM:REM + 3] = 1.0
        xr_c[REM + 3:2 * REM + 3] = xh_full[NFULL * 128:]
        xr_c[2 * REM + 3:] = xlr[sl].transpose(2, 1, 0)
        in_maps.append({
            "xh": xh_c, "xl": xl_c, "xht": xht_c, "xlt": xlt_c, "xr": xr_c,
            "wh": wh, "wl": wl, "w8": w8, "wrem": wrem, "w2": w2, "b2e": b2e,
        })
    return in_maps


def postprocess(results):
    spk = np.empty((T, B, NOUT), np.float32)
    mem = np.empty((T, B, NOUT), np.float32)
    for c, r in enumerate(results):
        sl = slice(c * BS, (c + 1) * BS)
        mem[:, sl, :] = r["m2out"].transpose(1, 2, 0)
        spk[:, sl, :] = r["s2out"].astype(np.float32).transpose(1, 2, 0)
    return spk, mem


_NC_CACHE = {}


def kernel(x, W1, b1, W2, b2):
    if "nc" not in _NC_CACHE:
        _NC_CACHE["nc"] = build_kernel(reps_loop=False)
    nc = _NC_CACHE["nc"]
    in_maps = prepare_inputs(x, W1, b1, W2, b2)
    res = bass_utils.run_bass_kernel_spmd(
        nc, in_maps, core_ids=list(range(N_CORES)))
    return postprocess(res.results)
